# revision 39
# baseline (speedup 1.0000x reference)
"""2-layer GCN (DBPnet GCN head) on 8 Trainium2 NeuronCores.

Algorithm (matches the jax reference):
    x0 = relu(x)
    x1 = relu(gcn_conv(x0, W1, b1))
    x2 = gcn_conv(x1, W2, b2)
    y  = softmax(x2, axis=-1)
with gcn_conv(x) = D^-1/2 (A + I) D^-1/2 (x @ W) + b  (in-degree over dst + 1).

Sharding: nodes row-partitioned over 8 cores; edges partitioned by
destination core so the segment-sum is core-local.  Destination nodes are
packed into variable-size windows (<=128 nodes, <=1024 edges per src-half)
so every (pass, window) is exactly 8 edge tiles on every core -- no
max-over-cores tile padding.  Nodes live in a padded slot space of
W*128 rows per core (pad slots carry zeros).

Per layer each core computes hs = dinv * (x_shard @ W), all-gathers hs into
a full bf16 table, gathers hs[src] rows for its edges with one 1024-idx
indirect DMA per (pass, window) cycled over the 4 SWDGE queues (descriptor
generation overlaps 4-wide), and segment-sums each window on the tensor
engine with a host-precomputed one-hot matrix S streamed from DRAM:
    psum_w = sum_tiles S_tile^T @ gathered_tile  (+ ident@acc + ident@hs)
    out_w  = act( dinv_w * psum_w [+ b] )
The decomposition dinv[src]*dinv[dst] = (pre-allgather) * (activation
scale) makes the per-edge norm free.  Layer 2's table is bf16 zero-padded
to 128 columns so its gather rows stay 256B and its matmuls run at bf16
rate.
"""

import sys

import numpy as np

sys.path.insert(0, "/opt/trn_rl_repo")

import ml_dtypes  # noqa: E402
from concourse import bass, mybir  # noqa: E402
import concourse.bacc as bacc  # noqa: E402
import concourse.tile as tile  # noqa: E402
from concourse.bass_utils import run_bass_kernel_spmd  # noqa: E402

F32 = mybir.dt.float32
BF16 = mybir.dt.bfloat16
I16 = mybir.dt.int16

C = 8            # cores
P = 128          # partitions / edge-tile size
TPW = 8          # edge tiles per (pass, window); gather = TPW*128 = 1024 idxs
PAD_SLOT = 200.0  # dst_slot value for padding edges (no onehot match)
ACT = mybir.ActivationFunctionType


# ---------------------------------------------------------------- host prep

def _schedule(src, dst, N):
    """Variable-window tile schedule.

    Packs each core's (node-ordered) destinations into windows of <=128
    nodes with <=TPW*128 edges per src-half, so every (pass, window) is
    exactly TPW tiles on every core.  Returns (W, per_core) with
    per_core[c] = (idx_wrapped [P, T*8] int16, slot [T*P] f32,
    w_of[NS], slot_of[NS]) and T = 2*W*TPW.
    """
    NS = N // C
    G4 = C // 2  # cores per src half
    CAP = TPW * P

    deg = [np.bincount(dst[(src >= h * (N // 2)) & (src < (h + 1) * (N // 2))],
                       minlength=N) for h in range(2)]

    wb = []       # per-core window start node (local), list per core
    w_of = []     # per-core local node -> window
    s_of = []     # per-core local node -> slot
    for c in range(C):
        d0 = deg[0][c * NS:(c + 1) * NS]
        d1 = deg[1][c * NS:(c + 1) * NS]
        wo = np.empty(NS, np.int32)
        so = np.empty(NS, np.int32)
        bounds = [0]
        i = 0
        w = 0
        while i < NS:
            s0 = s1 = n = 0
            while (i < NS and n < P and s0 + d0[i] <= CAP
                   and s1 + d1[i] <= CAP):
                wo[i] = w
                so[i] = n
                s0 += d0[i]
                s1 += d1[i]
                n += 1
                i += 1
            w += 1
            bounds.append(i)
        wb.append(bounds)
        w_of.append(wo)
        s_of.append(so)
    W = max(len(b) - 1 for b in wb)
    T = 2 * W * TPW
    NSP = W * P
    HALFR = G4 * NSP  # table rows per src half (padded space)

    # table row (within its half) for every global node
    owner = np.repeat(np.arange(C), NS)
    row_in_half = np.empty(N, np.int64)
    for c in range(C):
        loc = np.arange(NS)
        row_in_half[c * NS:(c + 1) * NS] = ((c % G4) * NSP
                                            + w_of[c][loc] * P + s_of[c][loc])
    src_h = (owner[src] >= G4).astype(np.int64)
    src_row = row_in_half[src]
    assert src_row.max() < 32768

    # global padded table row for every node (for the packed L2 gather)
    grow = np.empty(N, np.int64)
    for c in range(C):
        loc = np.arange(NS)
        grow[c * NS:(c + 1) * NS] = (c * NSP + w_of[c][loc] * P
                                     + s_of[c][loc])

    T2 = W * 2 * TPW  # L2: single pass, 16 tiles per window
    per_core = []
    for c in range(C):
        m = (dst >= c * NS) & (dst < (c + 1) * NS)
        e_src = src[m]
        e_dst = dst[m] - c * NS
        e_h0 = src_h[m]
        e_w0 = w_of[c][e_dst]
        e_slot0 = s_of[c][e_dst]
        e_row0 = src_row[m]
        e_grow0 = grow[e_src]

        # ---- layer-1 schedule: (src-half, window) two-pass -------------
        order = np.lexsort((e_w0, e_h0))
        e_h, e_w, e_slot, e_row = (e_h0[order], e_w0[order],
                                   e_slot0[order], e_row0[order])
        si = np.zeros(T * P, np.int16)
        sl = np.full(T * P, PAD_SLOT, np.float32)
        key = e_h * W + e_w
        cnt = np.bincount(key, minlength=2 * W)
        starts = np.concatenate([[0], np.cumsum(cnt)])[:-1]
        rank = np.arange(len(key)) - starts[key]
        assert rank.max() < CAP
        pos = key * CAP + rank
        si[pos] = e_row.astype(np.int16)
        sl[pos] = e_slot.astype(np.float32)
        siw = np.ascontiguousarray(np.tile(si.reshape(T * 8, 16).T, (8, 1)))

        # ---- layer-2 schedule: single pass over packed row pairs -------
        order2 = np.argsort(e_w0, kind="stable")
        e_w2, e_slot2, e_grow2 = (e_w0[order2], e_slot0[order2],
                                  e_grow0[order2])
        si2 = np.zeros(T2 * P, np.int16)
        sle = np.full(T2 * P, PAD_SLOT, np.float32)
        slo = np.full(T2 * P, PAD_SLOT, np.float32)
        cnt2 = np.bincount(e_w2, minlength=W)
        starts2 = np.concatenate([[0], np.cumsum(cnt2)])[:-1]
        rank2 = np.arange(len(e_w2)) - starts2[e_w2]
        assert rank2.max() < 2 * CAP
        pos2 = e_w2 * 2 * CAP + rank2
        par = (e_grow2 & 1).astype(np.float32)
        si2[pos2] = (e_grow2 >> 1).astype(np.int16)
        sle[pos2] = np.where(par == 0, e_slot2, PAD_SLOT).astype(np.float32)
        slo[pos2] = np.where(par == 1, e_slot2, PAD_SLOT).astype(np.float32)
        si2w = np.ascontiguousarray(
            np.tile(si2.reshape(T2 * 8, 16).T, (8, 1)))
        per_core.append((siw, sl, si2w, sle, slo, w_of[c], s_of[c]))
    return W, per_core


# ------------------------------------------------------------- device build

def build_program(nc, N, H, F1, F2, W, has_bias, cc=True):
    """Emit the SPMD program. All cores run identical code; per-core data
    comes in through the input tensors."""
    NSP = W * P          # padded node slots per core
    G4 = C // 2
    HALFR = G4 * NSP     # table rows per src half
    T = 2 * W * TPW
    F2P = P              # layer-2 table padded to 128 bf16 cols

    d_xT = nc.dram_tensor("xT", [H, NSP], BF16, kind="ExternalInput")
    d_W1 = nc.dram_tensor("W1", [H, F1], BF16, kind="ExternalInput")
    d_W2 = nc.dram_tensor("W2", [F1, F2], BF16, kind="ExternalInput")
    d_dinv = nc.dram_tensor("dinv", [P, W], F32, kind="ExternalInput")
    d_ident = nc.dram_tensor("ident", [P, P], BF16, kind="ExternalInput")
    T2 = W * 2 * TPW
    d_si = nc.dram_tensor("srcidx", [P, T * 8], I16, kind="ExternalInput")
    d_S = nc.dram_tensor("onehotS", [P, T * P], BF16, kind="ExternalInput")
    d_si2 = nc.dram_tensor("srcidx2", [P, T2 * 8], I16,
                           kind="ExternalInput")
    d_Se = nc.dram_tensor("onehotSe", [P, T2 * P], BF16,
                          kind="ExternalInput")
    d_So = nc.dram_tensor("onehotSo", [P, T2 * P], BF16,
                          kind="ExternalInput")
    if has_bias:
        d_b1 = nc.dram_tensor("b1r", [P, F1], F32, kind="ExternalInput")
        d_b2 = nc.dram_tensor("b2r", [P, F2], F32, kind="ExternalInput")
    d_y = nc.dram_tensor("y", [NSP, F2], F32, kind="ExternalOutput")

    with tile.TileContext(nc) as tc:
        with (
            tc.tile_pool(name="const", bufs=1) as const_pool,
            tc.tile_pool(name="persist", bufs=1) as persist,
            tc.tile_pool(name="gath", bufs=16) as gath_pool,
            tc.tile_pool(name="sload", bufs=6) as s_pool,
            tc.tile_pool(name="winbuf", bufs=3) as win_pool,
            tc.tile_pool(name="small", bufs=6) as small_pool,
            tc.tile_pool(name="agg", bufs=4, space="PSUM") as psum_agg,
            tc.tile_pool(name="dense", bufs=2, space="PSUM") as psum_dense,
            tc.tile_pool(name="tpose", bufs=2, space="PSUM") as psum_t,
            tc.tile_pool(name="dram", bufs=1, space="DRAM") as dram,
        ):
            # ---- constants / persistent state -----------------------------
            # order matters: the sync queue is FIFO, so xT (needed first by
            # the dense phase) is loaded before the large srcidx table
            sb_W1 = const_pool.tile([H, F1], BF16, tag="w1")
            nc.sync.dma_start(out=sb_W1[:], in_=d_W1[:])
            sb_W2 = const_pool.tile([F1, F2], BF16, tag="w2")
            nc.sync.dma_start(out=sb_W2[:], in_=d_W2[:])
            sb_dinv = const_pool.tile([P, W], F32, tag="dinv")
            nc.sync.dma_start(out=sb_dinv[:], in_=d_dinv[:])
            sb_ident = const_pool.tile([P, P], BF16, tag="ident")
            nc.sync.dma_start(out=sb_ident[:], in_=d_ident[:])
            if has_bias:
                sb_b1 = const_pool.tile([P, F1], F32, tag="b1")
                nc.sync.dma_start(out=sb_b1[:], in_=d_b1[:])
                sb_b2 = const_pool.tile([P, F2], F32, tag="b2")
                nc.sync.dma_start(out=sb_b2[:], in_=d_b2[:])

            sb_si = const_pool.tile([P, T * 8], I16, tag="srcidx")
            nc.sync.dma_start(out=sb_si[:], in_=d_si[:])
            sb_si2 = const_pool.tile([P, T2 * 8], I16, tag="srcidx2")
            nc.sync.dma_start(out=sb_si2[:], in_=d_si2[:])
            sb_xT = persist.tile([H, NSP], BF16, tag="xT")
            nc.sync.dma_start(out=sb_xT[:], in_=d_xT[:])
            sb_hs1 = persist.tile([P, W, F1], BF16, tag="hs1")
            sb_hs2 = persist.tile([P, W, F2], BF16, tag="hs2")
            # pass-0 partial aggregates; acc1 reuses the xT slot (dead then)
            sb_acc1 = persist.tile([P, W, F1], BF16, tag="xT")

            hs1_loc = dram.tile([NSP, F1], BF16, tag="hs1_loc")
            hs1_full = dram.tile([C * NSP, F1], BF16, tag="hs1_full",
                                 addr_space="Shared")
            # layer-2 table: unpadded 64-col rows; gathered as 256B pairs
            hs2_loc = dram.tile([NSP, F2], BF16, tag="hs2_loc")
            hs2_full = dram.tile([C * NSP // 2, 2 * F2], BF16,
                                 tag="hs2_full", addr_space="Shared")

            def allgather(loc, full):
                nc.gpsimd.collective_compute(
                    "AllGather", mybir.AluOpType.bypass,
                    replica_groups=[list(range(C))],
                    ins=[loc[:].opt()], outs=[full[:].opt()])

            def loc_write(loc, w, src_ap):
                nc.sync.dma_start(out=loc[w * P:(w + 1) * P, :], in_=src_ap)

            # ---- phase 1: x0 = relu(x); hs1 = dinv * (x0 @ W1) ------------
            nc.vector.tensor_scalar_max(sb_xT[:], sb_xT[:], 0.0)
            for w in range(W):
                ph = psum_dense.tile([P, F1], F32, tag="dense")
                nc.tensor.matmul(ph[:], lhsT=sb_xT[:, w * P:(w + 1) * P],
                                 rhs=sb_W1[:], start=True, stop=True)
                nc.scalar.activation(sb_hs1[:, w, :], ph[:], ACT.Identity,
                                     scale=sb_dinv[:, w:w + 1])
                loc_write(hs1_loc, w, sb_hs1[:, w, :])

            # ---- phase 2: all-gather layer-1 table ------------------------
            allgather(hs1_loc, hs1_full)

            # ---- edge aggregation (both layers) ---------------------------
            def edge_layer(table, F, acc_sb, self_sb, out_cb):
                """Two passes (one per src half); pass 0 parks the partial
                window sums in acc_sb (bf16), pass 1 adds acc+self via
                identity matmuls and calls out_cb on the finished psum."""
                sts = {}
                for t0 in range(0, T, TPW):  # streamed one-hot S tiles
                    s = s_pool.tile([P, TPW, P], BF16, tag="sload")
                    nc.sync.dma_start(
                        out=s[:],
                        in_=d_S[:, t0 * P:(t0 + TPW) * P].rearrange(
                            "p (n j) -> p n j", n=TPW))
                    sts[t0 // TPW] = s
                for h in range(2):
                    tab = table[h * HALFR:(h + 1) * HALFR, :]
                    for w in range(W):
                        t0w = (h * W + w) * TPW
                        g = gath_pool.tile([P, TPW, F], BF16, tag="gath")
                        nc.gpsimd.dma_gather(
                            g[:], tab, sb_si[:, t0w * 8:(t0w + TPW) * 8],
                            TPW * P, TPW * P, F, queue_num=w % 4)
                        pa = psum_agg.tile([P, F], F32, tag="agg")
                        s = sts[t0w // TPW]
                        for t in range(TPW):
                            nc.tensor.matmul(
                                pa[:], lhsT=s[:, t, :], rhs=g[:, t, :],
                                start=(t == 0),
                                stop=(h == 0 and t == TPW - 1))
                        if h == 0:
                            nc.vector.tensor_copy(acc_sb[:, w, :], pa[:])
                        else:
                            nc.tensor.matmul(pa[:], lhsT=sb_ident[:],
                                             rhs=acc_sb[:, w, :],
                                             start=False, stop=False)
                            nc.tensor.matmul(pa[:], lhsT=sb_ident[:],
                                             rhs=self_sb[:, w, :],
                                             start=False, stop=True)
                            out_cb(w, pa)

            # ---- layer-1 epilogue: relu, transpose, dense L2 --------------
            def l1_out(w, pa):
                x1 = win_pool.tile([P, F1], BF16, tag="x1")
                if has_bias:
                    tmp = win_pool.tile([P, F1], F32, tag="tmp")
                    nc.scalar.activation(tmp[:], pa[:], ACT.Identity,
                                         scale=sb_dinv[:, w:w + 1])
                    nc.vector.tensor_tensor(out=tmp[:], in0=tmp[:],
                                            in1=sb_b1[:],
                                            op=mybir.AluOpType.add)
                    nc.scalar.activation(x1[:], tmp[:], ACT.Relu)
                else:
                    nc.scalar.activation(x1[:], pa[:], ACT.Relu,
                                         scale=sb_dinv[:, w:w + 1])
                pt = psum_t.tile([P, P], BF16, tag="tpose")
                nc.tensor.transpose(pt[:], x1[:], sb_ident[:])
                x1T = win_pool.tile([P, P], BF16, tag="x1T")
                nc.vector.tensor_copy(x1T[:], pt[:])
                ph2 = psum_dense.tile([P, F2], F32, tag="dense")
                nc.tensor.matmul(ph2[:], lhsT=x1T[:], rhs=sb_W2[:],
                                 start=True, stop=True)
                nc.scalar.activation(sb_hs2[:, w, :], ph2[:], ACT.Identity,
                                     scale=sb_dinv[:, w:w + 1])
                loc_write(hs2_loc, w, sb_hs2[:, w, :])

            edge_layer(hs1_full, F1, sb_acc1, sb_hs1, l1_out)

            # ---- phase 5: all-gather layer-2 table ------------------------
            allgather(hs2_loc, hs2_full)

            # ---- phase 6: layer-2 edges + softmax -------------------------
            def l2_out(w, pa):
                ex = win_pool.tile([P, F2], F32, tag="ex")
                ssum = small_pool.tile([P, 1], F32, tag="ssum")
                if has_bias:
                    tmp = win_pool.tile([P, F2], F32, tag="tmp2")
                    nc.scalar.activation(tmp[:], pa[:], ACT.Identity,
                                         scale=sb_dinv[:, w:w + 1])
                    nc.vector.tensor_tensor(out=tmp[:], in0=tmp[:],
                                            in1=sb_b2[:],
                                            op=mybir.AluOpType.add)
                    nc.scalar.activation(ex[:], tmp[:], ACT.Exp,
                                         accum_out=ssum[:])
                else:
                    # logits are O(1) for this model family; exp without
                    # max-subtraction is safe in fp32
                    nc.scalar.activation(ex[:], pa[:], ACT.Exp,
                                         scale=sb_dinv[:, w:w + 1],
                                         accum_out=ssum[:])
                rsum = small_pool.tile([P, 1], F32, tag="rsum")
                nc.vector.reciprocal(rsum[:], ssum[:])
                yw = win_pool.tile([P, F2], F32, tag="yw")
                nc.vector.tensor_scalar_mul(yw[:], ex[:], rsum[:])
                nc.sync.dma_start(out=d_y[w * P:(w + 1) * P, :],
                                  in_=yw[:])

            # single pass over packed row pairs; parity selected by the
            # host-built S_even / S_odd one-hot matrices
            T16 = 2 * TPW
            for w in range(W):
                se = s_pool.tile([P, T16, P], BF16, tag="se")
                nc.sync.dma_start(
                    out=se[:],
                    in_=d_Se[:, w * T16 * P:(w + 1) * T16 * P].rearrange(
                        "p (n j) -> p n j", n=T16))
                so = s_pool.tile([P, T16, P], BF16, tag="so")
                nc.sync.dma_start(
                    out=so[:],
                    in_=d_So[:, w * T16 * P:(w + 1) * T16 * P].rearrange(
                        "p (n j) -> p n j", n=T16))
                gs = []
                for k in range(2):
                    g = gath_pool.tile([P, TPW, 2 * F2], BF16, tag="gath")
                    t0 = w * T16 + k * TPW
                    nc.gpsimd.dma_gather(
                        g[:], hs2_full[:],
                        sb_si2[:, t0 * 8:(t0 + TPW) * 8],
                        TPW * P, TPW * P, 2 * F2,
                        queue_num=(2 * w + k) % 4)
                    gs.append(g)
                pa = psum_agg.tile([P, F2], F32, tag="agg")
                for k in range(2):
                    for t in range(TPW):
                        tt = k * TPW + t
                        nc.tensor.matmul(pa[:], lhsT=se[:, tt, :],
                                         rhs=gs[k][:, t, :F2],
                                         start=(tt == 0), stop=False)
                        nc.tensor.matmul(pa[:], lhsT=so[:, tt, :],
                                         rhs=gs[k][:, t, F2:],
                                         start=False, stop=False)
                nc.tensor.matmul(pa[:], lhsT=sb_ident[:],
                                 rhs=sb_hs2[:, w, :],
                                 start=False, stop=True)
                l2_out(w, pa)

    in_names = ["xT", "W1", "W2", "dinv", "ident", "srcidx", "onehotS",
                "srcidx2", "onehotSe", "onehotSo"]
    if has_bias:
        in_names += ["b1r", "b2r"]
    return {"in_names": in_names, "out_name": "y"}


# ---------------------------------------------------------------- frontend

_CACHE = {}


def _build_and_compile(N, H, F1, F2, W, has_bias):
    nc = bacc.Bacc("TRN2", target_bir_lowering=False, debug=False,
                   enable_asserts=False, num_devices=C,
                   num_swdge_queues=4)
    meta = build_program(nc, N, H, F1, F2, W, has_bias)
    nc.compile()
    return nc, meta


def prepare_inputs(x, edge_index, W1, b1, W2, b2):
    N, H = x.shape
    F1 = W1.shape[1]
    F2 = W2.shape[1]
    NS = N // C

    src = np.asarray(edge_index[0], dtype=np.int64)
    dst = np.asarray(edge_index[1], dtype=np.int64)
    deg = np.bincount(dst, minlength=N).astype(np.float32) + 1.0
    dinv_n = (1.0 / np.sqrt(deg)).astype(np.float32)

    W, per_core = _schedule(src, dst, N)
    NSP = W * P
    T = 2 * W * TPW

    has_bias = bool(np.any(np.asarray(b1)) or np.any(np.asarray(b2)))
    ident = np.eye(P, dtype=ml_dtypes.bfloat16)
    W1h = np.asarray(W1, np.float32).astype(ml_dtypes.bfloat16)
    W2h = np.asarray(W2, np.float32).astype(ml_dtypes.bfloat16)
    if has_bias:
        b1r = np.ascontiguousarray(np.tile(np.asarray(b1, np.float32),
                                           (P, 1)))
        b2r = np.ascontiguousarray(np.tile(np.asarray(b2, np.float32),
                                           (P, 1)))

    jj = np.arange(P, dtype=np.float32)
    xf = np.asarray(x, np.float32)

    def onehot(sl, t):
        S = (sl.reshape(t, P)[:, :, None] ==
             jj[None, None, :]).astype(ml_dtypes.bfloat16)
        return np.ascontiguousarray(S.transpose(1, 0, 2).reshape(P, t * P))

    T2 = W * 2 * TPW
    in_maps = []
    slot_maps = []
    for c in range(C):
        si, sl, si2, sle, slo, w_of, s_of = per_core[c]
        pos = w_of.astype(np.int64) * P + s_of  # local node -> padded slot
        slot_maps.append(pos)
        xs = np.zeros((NSP, H), np.float32)
        xs[pos] = xf[c * NS:(c + 1) * NS]
        xT = np.ascontiguousarray(xs.T).astype(ml_dtypes.bfloat16)
        dv = np.ones(NSP, np.float32)
        dv[pos] = dinv_n[c * NS:(c + 1) * NS]
        dv = np.ascontiguousarray(dv.reshape(W, P).T)
        im = {
            "xT": xT, "W1": W1h, "W2": W2h, "dinv": dv, "ident": ident,
            "srcidx": si, "onehotS": onehot(sl, T),
            "srcidx2": si2, "onehotSe": onehot(sle, T2),
            "onehotSo": onehot(slo, T2),
        }
        if has_bias:
            im["b1r"] = b1r
            im["b2r"] = b2r
        in_maps.append(im)
    return in_maps, slot_maps, (N, H, F1, F2, W, has_bias)


def kernel(x, edge_index, W1, b1, W2, b2, trace=False):
    x = np.asarray(x)
    in_maps, slot_maps, key = prepare_inputs(x, edge_index, W1, b1, W2, b2)
    N, H, F1, F2, W, has_bias = key
    NS = N // C
    if key not in _CACHE:
        _CACHE.clear()
        _CACHE[key] = _build_and_compile(N, H, F1, F2, W, has_bias)
    nc, meta = _CACHE[key]
    res = run_bass_kernel_spmd(nc, in_maps, core_ids=list(range(C)),
                               trace=trace)
    y = np.empty((N, F2), np.float32)
    for c in range(C):
        y[c * NS:(c + 1) * NS] = res.results[c]["y"][slot_maps[c]]
    if trace:
        kernel.last_exec_time_ns = res.exec_time_ns
    return y.astype(np.float32)


kernel.last_exec_time_ns = None


# revision 44
# speedup vs baseline: 1.1858x; 1.1858x over previous
"""2-layer GCN (DBPnet GCN head) on 8 Trainium2 NeuronCores.

Algorithm (matches the jax reference):
    x0 = relu(x)
    x1 = relu(gcn_conv(x0, W1, b1))
    x2 = gcn_conv(x1, W2, b2)
    y  = softmax(x2, axis=-1)
with gcn_conv(x) = D^-1/2 (A + I) D^-1/2 (x @ W) + b  (in-degree over dst + 1).

Sharding: nodes row-partitioned over 8 cores; edges partitioned by
destination core so the segment-sum is core-local.  Destination nodes are
packed into variable-size windows (<=128 nodes, <=1024 edges per src-half)
so every (pass, window) is exactly 8 edge tiles on every core -- no
max-over-cores tile padding.  Nodes live in a padded slot space of
W*128 rows per core (pad slots carry zeros).

Per layer each core computes hs = dinv * (x_shard @ W), all-gathers hs into
a full bf16 table, gathers hs[src] rows for its edges with one 1024-idx
indirect DMA per (pass, window) cycled over the 4 SWDGE queues (descriptor
generation overlaps 4-wide), and segment-sums each window on the tensor
engine with a host-precomputed one-hot matrix S streamed from DRAM:
    psum_w = sum_tiles S_tile^T @ gathered_tile  (+ ident@acc + ident@hs)
    out_w  = act( dinv_w * psum_w [+ b] )
The decomposition dinv[src]*dinv[dst] = (pre-allgather) * (activation
scale) makes the per-edge norm free.  Layer 2's table is bf16 zero-padded
to 128 columns so its gather rows stay 256B and its matmuls run at bf16
rate.
"""

import sys

import numpy as np

sys.path.insert(0, "/opt/trn_rl_repo")

import ml_dtypes  # noqa: E402
from concourse import bass, mybir  # noqa: E402
import concourse.bacc as bacc  # noqa: E402
import concourse.tile as tile  # noqa: E402
from concourse.bass_utils import run_bass_kernel_spmd  # noqa: E402

F32 = mybir.dt.float32
BF16 = mybir.dt.bfloat16
I16 = mybir.dt.int16

C = 8            # cores
P = 128          # partitions / edge-tile size
TPW = 8          # edge tiles per (pass, window); gather = TPW*128 = 1024 idxs
PAD_SLOT = 200.0  # dst_slot value for padding edges (no onehot match)
ACT = mybir.ActivationFunctionType


# ---------------------------------------------------------------- host prep

def _schedule(src, dst, N):
    """Variable-window tile schedule.

    Packs each core's (node-ordered) destinations into windows of <=128
    nodes with <=TPW*128 edges per src-half, so every (pass, window) is
    exactly TPW tiles on every core.  Returns (W, per_core) with
    per_core[c] = (idx_wrapped [P, T*8] int16, slot [T*P] f32,
    w_of[NS], slot_of[NS]) and T = 2*W*TPW.
    """
    NS = N // C
    G4 = C // 2  # cores per src half
    CAP = TPW * P

    deg = [np.bincount(dst[(src >= h * (N // 2)) & (src < (h + 1) * (N // 2))],
                       minlength=N) for h in range(2)]

    wb = []       # per-core window start node (local), list per core
    w_of = []     # per-core local node -> window
    s_of = []     # per-core local node -> slot
    for c in range(C):
        d0 = deg[0][c * NS:(c + 1) * NS]
        d1 = deg[1][c * NS:(c + 1) * NS]
        wo = np.empty(NS, np.int32)
        so = np.empty(NS, np.int32)
        bounds = [0]
        i = 0
        w = 0
        while i < NS:
            s0 = s1 = n = 0
            while (i < NS and n < P and s0 + d0[i] <= CAP
                   and s1 + d1[i] <= CAP):
                wo[i] = w
                so[i] = n
                s0 += d0[i]
                s1 += d1[i]
                n += 1
                i += 1
            w += 1
            bounds.append(i)
        wb.append(bounds)
        w_of.append(wo)
        s_of.append(so)
    W = max(len(b) - 1 for b in wb)
    T = 2 * W * TPW
    NSP = W * P
    HALFR = G4 * NSP  # table rows per src half (padded space)

    # table row (within its half) for every global node
    owner = np.repeat(np.arange(C), NS)
    row_in_half = np.empty(N, np.int64)
    for c in range(C):
        loc = np.arange(NS)
        row_in_half[c * NS:(c + 1) * NS] = ((c % G4) * NSP
                                            + w_of[c][loc] * P + s_of[c][loc])
    src_h = (owner[src] >= G4).astype(np.int64)
    src_row = row_in_half[src]
    assert src_row.max() < 32768

    per_core = []
    for c in range(C):
        m = (dst >= c * NS) & (dst < (c + 1) * NS)
        e_src = src[m]
        e_dst = dst[m] - c * NS
        e_h = src_h[m]
        e_w = w_of[c][e_dst]
        e_slot = e_dst * 0 + s_of[c][e_dst]
        e_row = src_row[m]
        order = np.lexsort((e_h, e_w))
        e_h, e_w, e_slot, e_row = (e_h[order], e_w[order],
                                   e_slot[order], e_row[order])
        si = np.zeros(T * P, np.int16)
        sl = np.full(T * P, PAD_SLOT, np.float32)
        # window-major rank: tiles [w][h][t]
        key = e_w * 2 + e_h
        cnt = np.bincount(key, minlength=2 * W)
        starts = np.concatenate([[0], np.cumsum(cnt)])[:-1]
        rank = np.arange(len(key)) - starts[key]
        assert rank.max() < CAP
        pos = key * CAP + rank
        si[pos] = e_row.astype(np.int16)
        sl[pos] = e_slot.astype(np.float32)
        siw = np.ascontiguousarray(np.tile(si.reshape(T * 8, 16).T, (8, 1)))
        per_core.append((siw, sl, w_of[c], s_of[c]))
    return W, per_core


# ------------------------------------------------------------- device build

def build_program(nc, N, H, F1, F2, W, has_bias, cc=True):
    """Emit the SPMD program. All cores run identical code; per-core data
    comes in through the input tensors."""
    NSP = W * P          # padded node slots per core
    G4 = C // 2
    HALFR = G4 * NSP     # table rows per src half
    T = 2 * W * TPW
    F2P = P              # layer-2 table padded to 128 bf16 cols

    d_xT = nc.dram_tensor("xT", [H, NSP], BF16, kind="ExternalInput")
    d_W1 = nc.dram_tensor("W1", [H, F1], BF16, kind="ExternalInput")
    d_W2 = nc.dram_tensor("W2", [F1, F2], BF16, kind="ExternalInput")
    d_dinv = nc.dram_tensor("dinv", [P, W], F32, kind="ExternalInput")
    d_ident = nc.dram_tensor("ident", [P, P], BF16, kind="ExternalInput")
    d_si = nc.dram_tensor("srcidx", [P, T * 8], I16, kind="ExternalInput")
    d_S = nc.dram_tensor("onehotS", [P, T * P], BF16, kind="ExternalInput")
    if has_bias:
        d_b1 = nc.dram_tensor("b1r", [P, F1], F32, kind="ExternalInput")
        d_b2 = nc.dram_tensor("b2r", [P, F2], F32, kind="ExternalInput")
    d_y = nc.dram_tensor("y", [NSP, F2], F32, kind="ExternalOutput")

    with tile.TileContext(nc) as tc:
        with (
            tc.tile_pool(name="const", bufs=1) as const_pool,
            tc.tile_pool(name="persist", bufs=1) as persist,
            tc.tile_pool(name="gath", bufs=16) as gath_pool,
            tc.tile_pool(name="sload", bufs=6) as s_pool,
            tc.tile_pool(name="winbuf", bufs=3) as win_pool,
            tc.tile_pool(name="small", bufs=6) as small_pool,
            tc.tile_pool(name="agg", bufs=4, space="PSUM") as psum_agg,
            tc.tile_pool(name="dense", bufs=2, space="PSUM") as psum_dense,
            tc.tile_pool(name="tpose", bufs=2, space="PSUM") as psum_t,
            tc.tile_pool(name="dram", bufs=1, space="DRAM") as dram,
        ):
            # ---- constants / persistent state -----------------------------
            # order matters: the sync queue is FIFO, so xT (needed first by
            # the dense phase) is loaded before the large srcidx table
            sb_W1 = const_pool.tile([H, F1], BF16, tag="w1")
            nc.sync.dma_start(out=sb_W1[:], in_=d_W1[:])
            sb_W2 = const_pool.tile([F1, F2], BF16, tag="w2")
            nc.sync.dma_start(out=sb_W2[:], in_=d_W2[:])
            sb_dinv = const_pool.tile([P, W], F32, tag="dinv")
            nc.sync.dma_start(out=sb_dinv[:], in_=d_dinv[:])
            sb_ident = const_pool.tile([P, P], BF16, tag="ident")
            nc.sync.dma_start(out=sb_ident[:], in_=d_ident[:])
            if has_bias:
                sb_b1 = const_pool.tile([P, F1], F32, tag="b1")
                nc.sync.dma_start(out=sb_b1[:], in_=d_b1[:])
                sb_b2 = const_pool.tile([P, F2], F32, tag="b2")
                nc.sync.dma_start(out=sb_b2[:], in_=d_b2[:])

            sb_si = const_pool.tile([P, T * 8], I16, tag="srcidx")
            nc.sync.dma_start(out=sb_si[:], in_=d_si[:])
            sb_xT = persist.tile([H, NSP], BF16, tag="xT")
            nc.sync.dma_start(out=sb_xT[:], in_=d_xT[:])
            sb_hs1 = persist.tile([P, W, F1], BF16, tag="hs1")
            sb_hs2 = persist.tile([P, W, F2P], BF16, tag="hs2")
            # pass-0 partial aggregates; acc1 reuses the xT slot (dead then)

            hs1_loc = dram.tile([NSP, F1], BF16, tag="hs1_loc")
            hs1_full = dram.tile([C * NSP, F1], BF16, tag="hs1_full",
                                 addr_space="Shared")
            hs2_loc = dram.tile([NSP, F2P], BF16, tag="hs2_loc")
            hs2_full = dram.tile([C * NSP, F2P], BF16, tag="hs2_full",
                                 addr_space="Shared")

            def allgather(loc, full):
                nc.gpsimd.collective_compute(
                    "AllGather", mybir.AluOpType.bypass,
                    replica_groups=[list(range(C))],
                    ins=[loc[:].opt()], outs=[full[:].opt()])

            def loc_write(loc, w, src_ap):
                nc.sync.dma_start(out=loc[w * P:(w + 1) * P, :], in_=src_ap)

            # ---- phase 1: x0 = relu(x); hs1 = dinv * (x0 @ W1) ------------
            nc.vector.tensor_scalar_max(sb_xT[:], sb_xT[:], 0.0)
            nc.vector.memset(sb_hs2[:, :, F2:], 0.0)  # L2 table zero pad
            for w in range(W):
                ph = psum_dense.tile([P, F1], F32, tag="dense")
                nc.tensor.matmul(ph[:], lhsT=sb_xT[:, w * P:(w + 1) * P],
                                 rhs=sb_W1[:], start=True, stop=True)
                nc.scalar.activation(sb_hs1[:, w, :], ph[:], ACT.Identity,
                                     scale=sb_dinv[:, w:w + 1])
                loc_write(hs1_loc, w, sb_hs1[:, w, :])

            # ---- phase 2: all-gather layer-1 table ------------------------
            allgather(hs1_loc, hs1_full)

            # ---- edge aggregation (both layers) ---------------------------
            def edge_layer(table, F, self_sb, out_cb):
                """Window-major: per window, gather + matmul both src
                halves into one psum, add the self-loop term via an
                identity matmul, then hand the psum to out_cb."""
                sts = {}
                T16 = 2 * TPW
                for w in range(W):  # streamed one-hot S tiles, [w][h][t]
                    s = s_pool.tile([P, T16, P], BF16, tag="sload")
                    nc.sync.dma_start(
                        out=s[:],
                        in_=d_S[:, w * T16 * P:(w + 1) * T16 * P].rearrange(
                            "p (n j) -> p n j", n=T16))
                    sts[w] = s
                for w in range(W):
                    pa = psum_agg.tile([P, F], F32, tag="agg")
                    s = sts[w]
                    for h in range(2):
                        tab = table[h * HALFR:(h + 1) * HALFR, :]
                        t0w = (w * 2 + h) * TPW
                        g = gath_pool.tile([P, TPW, F], BF16, tag="gath")
                        nc.gpsimd.dma_gather(
                            g[:], tab, sb_si[:, t0w * 8:(t0w + TPW) * 8],
                            TPW * P, TPW * P, F,
                            queue_num=(2 * w + h) % 4)
                        for t in range(TPW):
                            nc.tensor.matmul(
                                pa[:], lhsT=s[:, h * TPW + t, :],
                                rhs=g[:, t, :],
                                start=(h == 0 and t == 0), stop=False)
                    nc.tensor.matmul(pa[:], lhsT=sb_ident[:],
                                     rhs=self_sb[:, w, :],
                                     start=False, stop=True)
                    out_cb(w, pa)

            # ---- layer-1 epilogue: relu, transpose, dense L2 --------------
            def l1_out(w, pa):
                x1 = win_pool.tile([P, F1], BF16, tag="x1")
                if has_bias:
                    tmp = win_pool.tile([P, F1], F32, tag="tmp")
                    nc.scalar.activation(tmp[:], pa[:], ACT.Identity,
                                         scale=sb_dinv[:, w:w + 1])
                    nc.vector.tensor_tensor(out=tmp[:], in0=tmp[:],
                                            in1=sb_b1[:],
                                            op=mybir.AluOpType.add)
                    nc.scalar.activation(x1[:], tmp[:], ACT.Relu)
                else:
                    nc.scalar.activation(x1[:], pa[:], ACT.Relu,
                                         scale=sb_dinv[:, w:w + 1])
                pt = psum_t.tile([P, P], BF16, tag="tpose")
                nc.tensor.transpose(pt[:], x1[:], sb_ident[:])
                x1T = win_pool.tile([P, P], BF16, tag="x1T")
                nc.vector.tensor_copy(x1T[:], pt[:])
                ph2 = psum_dense.tile([P, F2], F32, tag="dense")
                nc.tensor.matmul(ph2[:], lhsT=x1T[:], rhs=sb_W2[:],
                                 start=True, stop=True)
                nc.scalar.activation(sb_hs2[:, w, :F2], ph2[:], ACT.Identity,
                                     scale=sb_dinv[:, w:w + 1])
                loc_write(hs2_loc, w, sb_hs2[:, w, :])

            edge_layer(hs1_full, F1, sb_hs1, l1_out)

            # ---- phase 5: all-gather layer-2 table ------------------------
            allgather(hs2_loc, hs2_full)

            # ---- phase 6: layer-2 edges + softmax -------------------------
            def l2_out(w, pa):
                ex = win_pool.tile([P, F2], F32, tag="ex")
                ssum = small_pool.tile([P, 1], F32, tag="ssum")
                if has_bias:
                    tmp = win_pool.tile([P, F2], F32, tag="tmp2")
                    nc.scalar.activation(tmp[:], pa[:, :F2], ACT.Identity,
                                         scale=sb_dinv[:, w:w + 1])
                    nc.vector.tensor_tensor(out=tmp[:], in0=tmp[:],
                                            in1=sb_b2[:],
                                            op=mybir.AluOpType.add)
                    nc.scalar.activation(ex[:], tmp[:], ACT.Exp,
                                         accum_out=ssum[:])
                else:
                    # logits are O(1) for this model family; exp without
                    # max-subtraction is safe in fp32
                    nc.scalar.activation(ex[:], pa[:, :F2], ACT.Exp,
                                         scale=sb_dinv[:, w:w + 1],
                                         accum_out=ssum[:])
                rsum = small_pool.tile([P, 1], F32, tag="rsum")
                nc.vector.reciprocal(rsum[:], ssum[:])
                yw = win_pool.tile([P, F2], F32, tag="yw")
                nc.vector.tensor_scalar_mul(yw[:], ex[:], rsum[:])
                nc.sync.dma_start(out=d_y[w * P:(w + 1) * P, :],
                                  in_=yw[:])

            edge_layer(hs2_full, F2P, sb_hs2, l2_out)

    in_names = ["xT", "W1", "W2", "dinv", "ident", "srcidx", "onehotS"]
    if has_bias:
        in_names += ["b1r", "b2r"]
    return {"in_names": in_names, "out_name": "y"}


# ---------------------------------------------------------------- frontend

_CACHE = {}


def _build_and_compile(N, H, F1, F2, W, has_bias):
    nc = bacc.Bacc("TRN2", target_bir_lowering=False, debug=False,
                   enable_asserts=False, num_devices=C,
                   num_swdge_queues=4)
    meta = build_program(nc, N, H, F1, F2, W, has_bias)
    nc.compile()
    return nc, meta


def prepare_inputs(x, edge_index, W1, b1, W2, b2):
    N, H = x.shape
    F1 = W1.shape[1]
    F2 = W2.shape[1]
    NS = N // C

    src = np.asarray(edge_index[0], dtype=np.int64)
    dst = np.asarray(edge_index[1], dtype=np.int64)
    deg = np.bincount(dst, minlength=N).astype(np.float32) + 1.0
    dinv_n = (1.0 / np.sqrt(deg)).astype(np.float32)

    W, per_core = _schedule(src, dst, N)
    NSP = W * P
    T = 2 * W * TPW

    has_bias = bool(np.any(np.asarray(b1)) or np.any(np.asarray(b2)))
    ident = np.eye(P, dtype=ml_dtypes.bfloat16)
    W1h = np.asarray(W1, np.float32).astype(ml_dtypes.bfloat16)
    W2h = np.asarray(W2, np.float32).astype(ml_dtypes.bfloat16)
    if has_bias:
        b1r = np.ascontiguousarray(np.tile(np.asarray(b1, np.float32),
                                           (P, 1)))
        b2r = np.ascontiguousarray(np.tile(np.asarray(b2, np.float32),
                                           (P, 1)))

    jj = np.arange(P, dtype=np.float32)
    xf = np.asarray(x, np.float32)
    in_maps = []
    slot_maps = []
    for c in range(C):
        si, sl, w_of, s_of = per_core[c]
        pos = w_of.astype(np.int64) * P + s_of  # local node -> padded slot
        slot_maps.append(pos)
        xs = np.zeros((NSP, H), np.float32)
        xs[pos] = xf[c * NS:(c + 1) * NS]
        xT = np.ascontiguousarray(xs.T).astype(ml_dtypes.bfloat16)
        dv = np.ones(NSP, np.float32)
        dv[pos] = dinv_n[c * NS:(c + 1) * NS]
        dv = np.ascontiguousarray(dv.reshape(W, P).T)
        S = (sl.reshape(T, P)[:, :, None] ==
             jj[None, None, :]).astype(ml_dtypes.bfloat16)
        S = np.ascontiguousarray(S.transpose(1, 0, 2).reshape(P, T * P))
        im = {
            "xT": xT, "W1": W1h, "W2": W2h, "dinv": dv, "ident": ident,
            "srcidx": si, "onehotS": S,
        }
        if has_bias:
            im["b1r"] = b1r
            im["b2r"] = b2r
        in_maps.append(im)
    return in_maps, slot_maps, (N, H, F1, F2, W, has_bias)


def kernel(x, edge_index, W1, b1, W2, b2, trace=False):
    x = np.asarray(x)
    in_maps, slot_maps, key = prepare_inputs(x, edge_index, W1, b1, W2, b2)
    N, H, F1, F2, W, has_bias = key
    NS = N // C
    if key not in _CACHE:
        _CACHE.clear()
        _CACHE[key] = _build_and_compile(N, H, F1, F2, W, has_bias)
    nc, meta = _CACHE[key]
    res = run_bass_kernel_spmd(nc, in_maps, core_ids=list(range(C)),
                               trace=trace)
    y = np.empty((N, F2), np.float32)
    for c in range(C):
        y[c * NS:(c + 1) * NS] = res.results[c]["y"][slot_maps[c]]
    if trace:
        kernel.last_exec_time_ns = res.exec_time_ns
    return y.astype(np.float32)


kernel.last_exec_time_ns = None
